# revision 4
# baseline (speedup 1.0000x reference)
"""Trainium2 Bass kernel for the CJEPA recurrent slot model.

Full-input contract: kernel(**inputs) takes the complete (unsharded) numpy
arrays and returns the full (B, T, N, D) output. Internally the batch is
sharded 4-per-core across 8 NeuronCores; the small parameter set is
replicated.

Per-core plan:
  Phase 1 (parallel over time): z = tanh(obs @ W_enc.T + b_enc),
  keys/queries, sigmoid attention, slot blend, L2 normalize (Newton rsqrt
  on DVE), pre-scaled by ALPHA; written to a DRAM scratch laid out
  [t][i=(b,n)][d] so phase 2 can stream per-step contiguous blocks.
  Phase 2 (sequential over T): transposed-layout recurrence
  S_t = ALPHA*Shat_t + (1-ALPHA)*tanh(W_temporal S_{t-1}) with four
  128x128 bf16 matmuls per step; PE transposes restore natural layout for
  the output DMA.
"""

from contextlib import ExitStack

import numpy as np

B, T_FULL, D_OBS, D, NV = 32, 256, 1024, 256, 16
N_CORES = 8
B_LOC = B // N_CORES        # 4
I_DIM = B_LOC * NV          # 64 recurrent sequences per core
ALPHA = 0.7

_CACHE = {}


def build(T=T_FULL):
    import concourse.tile as tile
    from concourse import bacc, masks, mybir

    F32 = mybir.dt.float32
    BF = mybir.dt.bfloat16
    I32 = mybir.dt.int32
    AF = mybir.ActivationFunctionType
    OP = mybir.AluOpType

    n_chunks = T // 32

    nc = bacc.Bacc("TRN2", target_bir_lowering=False, debug=False,
                   num_devices=N_CORES)
    obs_v = nc.dram_tensor("observations", [B_LOC, T, D_OBS], F32,
                           kind="ExternalInput").ap()
    wenc_v = nc.dram_tensor("W_enc", [D, D_OBS], F32,
                            kind="ExternalInput").ap()
    benc_v = nc.dram_tensor("b_enc", [D, 1], F32, kind="ExternalInput").ap()
    wkey_v = nc.dram_tensor("W_key", [D, D], F32, kind="ExternalInput").ap()
    wval_v = nc.dram_tensor("W_value", [D, D], F32,
                            kind="ExternalInput").ap()
    wqry_v = nc.dram_tensor("W_query", [NV, D, D], F32,
                            kind="ExternalInput").ap()
    bqry_v = nc.dram_tensor("b_query", [1, NV * D], F32,
                            kind="ExternalInput").ap()
    wtmp_v = nc.dram_tensor("W_temporal", [D, D], F32,
                            kind="ExternalInput").ap()
    out_v = nc.dram_tensor("out", [B_LOC, T, NV, D], F32,
                           kind="ExternalOutput").ap()

    with tile.TileContext(nc) as tc, ExitStack() as ctx:
        const = ctx.enter_context(tc.tile_pool(name="const", bufs=1))
        wpool = ctx.enter_context(tc.tile_pool(name="wpool", bufs=1))
        wtmp_pool = ctx.enter_context(tc.tile_pool(name="wtmp", bufs=2))
        p1 = ctx.enter_context(tc.tile_pool(name="p1", bufs=2))
        qpool = ctx.enter_context(tc.tile_pool(name="qpool", bufs=18))
        jpool = ctx.enter_context(tc.tile_pool(name="jpool", bufs=4))
        small = ctx.enter_context(tc.tile_pool(name="small", bufs=8))
        p2 = ctx.enter_context(tc.tile_pool(name="p2", bufs=4))
        dramp = ctx.enter_context(tc.tile_pool(name="dramp", bufs=1,
                                               space="DRAM"))
        ps1 = ctx.enter_context(tc.tile_pool(name="ps1", bufs=2,
                                             space="PSUM"))
        ps2 = ctx.enter_context(tc.tile_pool(name="ps2", bufs=2,
                                             space="PSUM"))
        ps3 = ctx.enter_context(tc.tile_pool(name="ps3", bufs=2,
                                             space="PSUM"))
        ps4 = ctx.enter_context(tc.tile_pool(name="ps4", bufs=2,
                                             space="PSUM"))

        scratch = dramp.tile([T, I_DIM, D], BF, tag="scratch")

        ident = const.tile([128, 128], BF, tag="ident")
        masks.make_identity(nc, ident[:])
        ones1 = const.tile([1, 128], BF, tag="ones1")
        nc.vector.memset(ones1[:], 1.0)

        benc = []
        for h in range(2):
            t_ = const.tile([128, 1], F32, tag=f"benc{h}")
            nc.sync.dma_start(t_[:], benc_v[h * 128:(h + 1) * 128, :])
            benc.append(t_)

        bq_f = const.tile([1, NV * D], F32, tag="bq_f")
        nc.sync.dma_start(bq_f[:], bqry_v[:])
        bq_bf = const.tile([1, NV * D], BF, tag="bq_bf")
        nc.vector.tensor_copy(bq_bf[:], bq_f[:])

        def copy_ps(dst, src, use_act):
            if use_act:
                nc.scalar.copy(dst, src)
            else:
                nc.vector.tensor_copy(dst, src)

        def prep_wT(dram_ap, rows, cols, name):
            """dram (rows=k, cols=d) f32 -> list over d-chunks j of bf16
            tiles (128, rows) holding W.T chunks."""
            cj = cols // 128
            rj = rows // 128
            wT = [wpool.tile([128, rows], BF, tag=f"{name}_T{j}",
                             name=f"{name}_T{j}")
                  for j in range(cj)]
            for rc in range(rj):
                nat = wtmp_pool.tile([128, cols], F32, tag="w_nat")
                nc.sync.dma_start(nat[:], dram_ap[rc * 128:(rc + 1) * 128, :])
                natb = wtmp_pool.tile([128, cols], BF, tag="w_natb")
                nc.vector.tensor_copy(natb[:], nat[:])
                for j in range(cj):
                    ps = ps1.tile([128, 128], BF, tag="t1")
                    nc.tensor.transpose(ps[:], natb[:, j * 128:(j + 1) * 128],
                                        ident[:])
                    copy_ps(wT[j][:, rc * 128:(rc + 1) * 128], ps[:],
                            use_act=(j % 2 == 0))
            return wT

        wencT = prep_wT(wenc_v, D, D_OBS, "enc")      # 8 x (128, 256)
        wkeyT = prep_wT(wkey_v, D, D, "key")          # 2 x (128, 256)
        wvalT = prep_wT(wval_v, D, D, "val")
        wtT = prep_wT(wtmp_v, D, D, "tmp")
        wqT = [prep_wT(wqry_v[n], D, D, f"q{n}") for n in range(NV)]

        def newton_rsqrt07(ss):
            """(128,16) f32 sum-of-squares -> ALPHA/max(sqrt(ss),1e-8)."""
            ssc = small.tile([128, NV], F32, tag="nw")
            nc.vector.tensor_scalar(ssc[:], ss[:], 1e-16, None, op0=OP.max)
            sh = small.tile([128, NV], I32, tag="nwi")
            nc.vector.tensor_scalar(sh[:], ssc[:].bitcast(I32), 1, None,
                                    op0=OP.logical_shift_right)
            yi = small.tile([128, NV], I32, tag="nwi")
            nc.vector.tensor_scalar(yi[:], sh[:], -1, 0x5F3759DF,
                                    op0=OP.mult, op1=OP.add)
            y = yi[:].bitcast(F32)
            for it in range(3):
                t1 = small.tile([128, NV], F32, tag="nw")
                nc.vector.tensor_tensor(t1[:], y, y, op=OP.mult)
                t2 = small.tile([128, NV], F32, tag="nw")
                nc.vector.scalar_tensor_tensor(t2[:], in0=t1[:], scalar=-0.5,
                                               in1=ssc[:], op0=OP.mult,
                                               op1=OP.mult)
                t3 = small.tile([128, NV], F32, tag="nw")
                nc.vector.tensor_scalar(t3[:], t2[:], 1.5, None, op0=OP.add)
                if it < 2:
                    yn = small.tile([128, NV], F32, tag="nw")
                    nc.vector.tensor_tensor(yn[:], y, t3[:], op=OP.mult)
                    y = yn[:]
                else:
                    rn = small.tile([128, NV], F32, tag="rn")
                    nc.vector.scalar_tensor_tensor(rn[:], in0=t3[:],
                                                   scalar=ALPHA, in1=y,
                                                   op0=OP.mult, op1=OP.mult)
            return rn

        scratch_flat = scratch[:].rearrange("t i d -> t (i d)")

        def phase1_chunk(c):
            obs_nat = p1.tile([128, D_OBS], F32, tag="obs_nat")
            for b in range(B_LOC):
                nc.sync.dma_start(obs_nat[b * 32:(b + 1) * 32, :],
                                  obs_v[b, c * 32:(c + 1) * 32, :])
            obs_bf = p1.tile([128, D_OBS], BF, tag="obs_bf")
            nc.vector.tensor_copy(obs_bf[:], obs_nat[:])

            obsT = []
            for j in range(8):
                ps = ps1.tile([128, 128], BF, tag="t1")
                nc.tensor.transpose(ps[:], obs_bf[:, j * 128:(j + 1) * 128],
                                    ident[:])
                ot = p1.tile([128, 128], BF, tag=f"obsT{j}")
                copy_ps(ot[:], ps[:], use_act=(j % 2 == 0))
                obsT.append(ot)

            zT = []
            for h in range(2):
                zp = ps2.tile([128, 128], F32, tag="t2")
                for j in range(8):
                    nc.tensor.matmul(zp[:],
                                     lhsT=wencT[j][:, h * 128:(h + 1) * 128],
                                     rhs=obsT[j][:], start=(j == 0),
                                     stop=(j == 7))
                zt = p1.tile([128, 128], BF, tag=f"zT{h}")
                nc.scalar.activation(zt[:], zp[:], AF.Tanh,
                                     bias=benc[h][:, 0:1])
                zT.append(zt)

            kv = {}
            for nm, wT in (("K", wkeyT), ("V", wvalT)):
                ps = ps3.tile([128, D], F32, tag="t3")
                for h in range(2):
                    nc.tensor.matmul(ps[:], lhsT=zT[h][:], rhs=wT[h][:],
                                     start=(h == 0), stop=(h == 1))
                t_ = p1.tile([128, D], BF, tag=f"{nm}_bf")
                nc.vector.tensor_copy(t_[:], ps[:])
                kv[nm] = t_

            logits = small.tile([128, NV], F32, tag="logits")
            q_tiles = []
            for n in range(NV):
                qp = ps3.tile([128, D], F32, tag="t3")
                nc.tensor.matmul(qp[:], lhsT=zT[0][:], rhs=wqT[n][0][:],
                                 start=True, stop=False)
                nc.tensor.matmul(qp[:], lhsT=zT[1][:], rhs=wqT[n][1][:],
                                 start=False, stop=False)
                nc.tensor.matmul(qp[:], lhsT=ones1[:],
                                 rhs=bq_bf[0:1, n * D:(n + 1) * D],
                                 start=False, stop=True)
                qt = qpool.tile([128, D], BF, tag="q_bf")
                nc.scalar.copy(qt[:], qp[:])
                junk = jpool.tile([128, D], BF, tag="junk")
                nc.vector.scalar_tensor_tensor(
                    junk[:], in0=qt[:], scalar=1.0 / 16.0, in1=kv["K"][:],
                    op0=OP.mult, op1=OP.mult,
                    accum_out=logits[:, n:n + 1])
                q_tiles.append(qt)

            attn = small.tile([128, NV], F32, tag="attn")
            nc.scalar.activation(attn[:], logits[:], AF.Sigmoid)

            ss = small.tile([128, NV], F32, tag="ss")
            shat_all = p1.tile([128, NV * D], BF, tag="shat_all")
            for n in range(NV):
                vmq = jpool.tile([128, D], BF, tag="vmq")
                nc.vector.tensor_tensor(vmq[:], kv["V"][:], q_tiles[n][:],
                                        op=OP.subtract)
                sh = shat_all[:, n * D:(n + 1) * D]
                nc.vector.scalar_tensor_tensor(sh, in0=vmq[:],
                                               scalar=attn[:, n:n + 1],
                                               in1=q_tiles[n][:],
                                               op0=OP.mult, op1=OP.add)
                junk2 = jpool.tile([128, D], BF, tag="junk")
                nc.scalar.activation(junk2[:], sh, AF.Square,
                                     accum_out=ss[:, n:n + 1])

            rn = newton_rsqrt07(ss)
            shat_fin = p1.tile([128, NV * D], BF, tag="shat_fin")
            for n in range(NV):
                nc.vector.tensor_scalar(shat_fin[:, n * D:(n + 1) * D],
                                        shat_all[:, n * D:(n + 1) * D],
                                        rn[:, n:n + 1], None, op0=OP.mult)
            for b in range(B_LOC):
                nc.sync.dma_start(
                    scratch_flat[c * 32:(c + 1) * 32,
                                 b * NV * D:(b + 1) * NV * D],
                    shat_fin[b * 32:(b + 1) * 32, :])

        state = {"prev": None}

        def phase2_step(t):
            sn = p2.tile([I_DIM, D], BF, tag="shat_nat")
            nc.sync.dma_start(sn[:], scratch[t, :, :])
            sp = ps4.tile([128, 128], BF, tag="t4")
            nc.tensor.transpose(sp[:, 0:64], sn[:, 0:128], ident[:64, :64])
            nc.tensor.transpose(sp[:, 64:128], sn[:, 128:256],
                                ident[:64, :64])
            if t == 0:
                s_bf = p2.tile([128, 128], BF, tag="state")
                nc.vector.tensor_scalar(s_bf[:], sp[:], 1.0 / ALPHA, None,
                                        op0=OP.mult)
            else:
                tp = ps2.tile([128, 128], F32, tag="t2")
                prev = state["prev"]
                for h in range(2):
                    for j in range(2):
                        nc.tensor.matmul(
                            tp[:, h * 64:(h + 1) * 64],
                            lhsT=wtT[j][:, h * 128:(h + 1) * 128],
                            rhs=prev[:, j * 64:(j + 1) * 64],
                            start=(j == 0), stop=(j == 1))
                th = p2.tile([128, 128], F32, tag="tanh")
                nc.scalar.activation(th[:], tp[:], AF.Tanh)
                s_bf = p2.tile([128, 128], BF, tag="state")
                nc.vector.scalar_tensor_tensor(s_bf[:], in0=th[:],
                                               scalar=1.0 - ALPHA,
                                               in1=sp[:], op0=OP.mult,
                                               op1=OP.add)
            onp = ps1.tile([I_DIM, D], BF, tag="t1")
            nc.tensor.transpose(onp[:, 0:128], s_bf[:, 0:64], ident[:])
            nc.tensor.transpose(onp[:, 128:256], s_bf[:, 64:128], ident[:])
            osb = p2.tile([I_DIM, D], F32, tag="out_sb")
            nc.scalar.copy(osb[:], onp[:])
            for b in range(B_LOC):
                nc.sync.dma_start(out_v[b, t, :, :],
                                  osb[b * NV:(b + 1) * NV, :])
            state["prev"] = s_bf

        for c in range(n_chunks):
            phase1_chunk(c)
        for t in range(T):
            phase2_step(t)

    nc.compile()
    return nc


def _get_nc():
    if "nc" not in _CACHE:
        _CACHE["nc"] = build(T_FULL)
    return _CACHE["nc"]


def kernel(observations, W_enc, b_enc, W_key, W_value, W_query, b_query,
           W_temporal):
    from concourse.bass_utils import run_bass_kernel_spmd

    nc = _get_nc()
    common = {
        "W_enc": np.ascontiguousarray(W_enc, np.float32),
        "b_enc": np.ascontiguousarray(b_enc, np.float32).reshape(D, 1),
        "W_key": np.ascontiguousarray(W_key, np.float32),
        "W_value": np.ascontiguousarray(W_value, np.float32),
        "W_query": np.ascontiguousarray(W_query, np.float32),
        "b_query": np.ascontiguousarray(b_query, np.float32).reshape(1, NV * D),
        "W_temporal": np.ascontiguousarray(W_temporal, np.float32),
    }
    obs = np.ascontiguousarray(observations, np.float32)
    in_maps = [
        dict(common,
             observations=np.ascontiguousarray(obs[c * B_LOC:(c + 1) * B_LOC]))
        for c in range(N_CORES)
    ]
    res = run_bass_kernel_spmd(nc, in_maps, list(range(N_CORES)))
    out = np.empty((B, T_FULL, NV, D), np.float32)
    for c in range(N_CORES):
        out[c * B_LOC:(c + 1) * B_LOC] = res.results[c]["out"]
    return out
